# revision 34
# baseline (speedup 1.0000x reference)
"""Multi-head attention + residual + LayerNorm on 8 TRN2 NeuronCores.

Problem shapes (hardcoded): bs=4, seq=1024, d_model=1024, head=16, dk=64.

Sharding (data parallel over (batch, query-token-half)):
  core c -> batch b = c // 2, query rows [512*(c%2), 512*(c%2)+512).
  Each core computes K/V projections for its batch's full 1024 tokens,
  Q projection for its 512 rows, attention for all 16 heads over its
  512 query rows, and residual+LayerNorm for those rows. No collectives.

Device kernel structure (single fused pipeline):
  - Host feeds q/k/v/W pre-transposed bf16 (contraction dim on
    partitions); all input DMAs are plain contiguous loads, ordered so
    the q/k projections can start as early as possible.
  - Main loop interleaves projection o-tiles with score+softmax work for
    the previous head pair, so ScalarE's exp stream hides under the
    PE's projection matmuls and the PE never idles long enough to lose
    the HAM clock boost.
  - Biases are fused into the PSUM->SBUF copies on the DVE (per-
    partition tensor_scalar for qp^T/kp^T, broadcast tensor_tensor for
    vp) - no PE bias matmuls.
  - scores S = qh^T.T @ kh^T in natural [qt, kt] layout, head pairs
    row-packed on the PE (partitions 0-63 / 64-127); exp on ScalarE
    (scale=1/8 folded, denominator via accum_out); normalize on DVE;
    att written to DRAM bf16 via SWDGE (gpsimd).
  - att read back TRANSPOSED via DMA-transpose in 4-head groups
    ([2048 x 128] -> [128 x 2048], 32 ops, sync ring only), issued as
    soon as each group's att is written so the readbacks overlap the
    main loop; the att.V matmuls run as one dense batch at the end,
    PE-transposed back to natural, then residual + LayerNorm (fp32).
"""

import numpy as np
import ml_dtypes

import concourse.bacc as bacc
import concourse.bass as bass
import concourse.mybir as mybir
import concourse.tile as tile
from concourse.masks import make_identity
from concourse.tile import add_dep_helper
from concourse.bass_utils import run_bass_kernel_spmd

P = 128
BS = 4
SEQ = 1024
D = 1024
H = 16
DK = 64
SQ = 512          # query rows per core
SK = SEQ          # kv rows per core
KD = D // P       # 8 contraction tiles
QT = SQ // P      # 4 query-row tiles
KT = SK // P      # 8 kv-row tiles
OT = D // P       # 8 output-feature tiles
HG = 4            # heads per readback group
EPS = 1e-5

BF = mybir.dt.bfloat16
F32 = mybir.dt.float32
AF = mybir.ActivationFunctionType

N_CORES = 8
BF_NP = ml_dtypes.bfloat16


def _emit(nc):
    """Emit the per-core Tile program."""
    qT_d = nc.dram_tensor("qT", (D, SQ), BF, kind="ExternalInput").ap()
    kT_d = nc.dram_tensor("kT", (D, SK), BF, kind="ExternalInput").ap()
    vT_d = nc.dram_tensor("vT", (D, SK), BF, kind="ExternalInput").ap()
    wqT_d = nc.dram_tensor("wqT", (D, D), BF, kind="ExternalInput").ap()
    wkT_d = nc.dram_tensor("wkT", (D, D), BF, kind="ExternalInput").ap()
    wvT_d = nc.dram_tensor("wvT", (D, D), BF, kind="ExternalInput").ap()
    bq = nc.dram_tensor("bq", (1, D), F32, kind="ExternalInput").ap()
    bk = nc.dram_tensor("bk", (1, D), F32, kind="ExternalInput").ap()
    bv = nc.dram_tensor("bv", (1, D), F32, kind="ExternalInput").ap()
    q_f32 = nc.dram_tensor("q_f32", (SQ, D), F32, kind="ExternalInput").ap()
    gamma = nc.dram_tensor("gamma", (1, D), F32, kind="ExternalInput").ap()
    beta = nc.dram_tensor("beta", (1, D), F32, kind="ExternalInput").ap()

    att_d = nc.dram_tensor("att", (H, SQ, SK), BF, kind="ExternalOutput").ap()
    normed_d = nc.dram_tensor("normed", (SQ, D), F32, kind="ExternalOutput").ap()

    def part3(ap):
        # (KD*P, F) dram view -> [p, kd, F] AP for a single big DMA
        return ap.rearrange("(kd p) f -> p kd f", p=P)

    with tile.TileContext(nc) as tc:
        with (
            tc.tile_pool(name="consts", bufs=1) as consts,
            tc.tile_pool(name="proj", bufs=1) as proj,
        ):
            ps_pool = [tc.alloc_tile_pool(name="psA", bufs=2, space="PSUM")]
            # ---------------- constants (gpsimd queue, tiny) ----------------
            ident_bf = consts.tile([P, P], BF)
            make_identity(nc, ident_bf)
            gamma_b = consts.tile([P, D], F32)
            nc.gpsimd.dma_start(out=gamma_b, in_=gamma.to_broadcast((P, D)))
            beta_b = consts.tile([P, D], F32)
            nc.gpsimd.dma_start(out=beta_b, in_=beta.to_broadcast((P, D)))
            bv_bc = consts.tile([P, D], BF)
            nc.gpsimd.dma_start(out=bv_bc, in_=bv.to_broadcast((P, D)))
            eps_t = consts.tile([P, 1], F32)
            nc.vector.memset(eps_t, EPS)
            # per-partition bias columns: bq_col[p, i] = bq[i*128 + p]
            bq_col = consts.tile([P, KD], F32)
            nc.gpsimd.dma_start(out=bq_col, in_=bq.rearrange("a (i p) -> p (a i)", p=P))
            bk_col = consts.tile([P, KD], F32)
            nc.gpsimd.dma_start(out=bk_col, in_=bk.rearrange("a (i p) -> p (a i)", p=P))

            # ---------------- input loads (pre-transposed on host) ----------
            # ordered so qp/kp (and the first score matmuls) start earliest
            with tc.tile_pool(name="inputs", bufs=1) as inp:
                wqT = inp.tile([P, KD, D], BF)
                wkT = inp.tile([P, KD, D], BF)
                wvT = inp.tile([P, KD, D], BF)
                kTt = inp.tile([P, KD, SK], BF)
                vTt = inp.tile([P, KD, SK], BF)
                qTt = inp.tile([P, KD, SQ], BF)
                nc.sync.dma_start(out=qTt, in_=part3(qT_d))
                nc.scalar.dma_start(out=wqT[:, :, :512], in_=part3(wqT_d)[:, :, :512])
                nc.sync.dma_start(out=kTt, in_=part3(kT_d))
                nc.scalar.dma_start(out=wkT[:, :, :512], in_=part3(wkT_d)[:, :, :512])
                nc.scalar.dma_start(out=wvT[:, :, :512], in_=part3(wvT_d)[:, :, :512])
                nc.sync.dma_start(out=vTt, in_=part3(vT_d))
                nc.scalar.dma_start(out=wqT[:, :, 512:], in_=part3(wqT_d)[:, :, 512:])
                nc.scalar.dma_start(out=wkT[:, :, 512:], in_=part3(wkT_d)[:, :, 512:])
                nc.scalar.dma_start(out=wvT[:, :, 512:], in_=part3(wvT_d)[:, :, 512:])

                qpT = proj.tile([P, OT, SQ], BF)  # qp^T[o, qt]
                kpT = proj.tile([P, OT, SK], BF)  # kp^T[o, kt]
                vp = proj.tile([P, KT, D], BF)    # vp[kt, o]

                with (
                    tc.tile_pool(name="p_sb", bufs=4) as p_pool,
                    tc.tile_pool(name="att_sb", bufs=3) as att_pool,
                    tc.tile_pool(name="attT", bufs=9) as attT_pool,
                    tc.tile_pool(name="den", bufs=12) as den_pool,
                    tc.tile_pool(name="oTb", bufs=2) as oTb_pool,
                    tc.tile_pool(name="ln", bufs=1) as ln_pool,
                ):
                    out_nat = proj.tile([P, QT, D], BF)

                    def emit_qp(i):
                        osl = slice(i * P, (i + 1) * P)
                        ps_q = ps_pool[0].tile([P, 512], F32, tag="pj", bufs=2, name="ps_q")
                        for kd in range(KD):
                            nc.tensor.matmul(
                                ps_q,
                                lhsT=wqT[:, kd, osl],
                                rhs=qTt[:, kd, :],
                                start=(kd == 0),
                                stop=(kd == KD - 1),
                            )
                        nc.vector.tensor_scalar_add(
                            out=qpT[:, i, :],
                            in0=ps_q,
                            scalar1=bq_col[:, i : i + 1],
                        )

                    def emit_kp(i, j):
                        osl = slice(i * P, (i + 1) * P)
                        jsl = slice(j * 512, (j + 1) * 512)
                        ps_k = ps_pool[0].tile([P, 512], F32, tag="pj", bufs=2, name="ps_k")
                        for kd in range(KD):
                            nc.tensor.matmul(
                                ps_k,
                                lhsT=wkT[:, kd, osl],
                                rhs=kTt[:, kd, jsl],
                                start=(kd == 0),
                                stop=(kd == KD - 1),
                            )
                        nc.vector.tensor_scalar_add(
                            out=kpT[:, i, jsl],
                            in0=ps_k,
                            scalar1=bk_col[:, i : i + 1],
                        )

                    def emit_vp(i, j):
                        osl = slice(i * P, (i + 1) * P)
                        jsl = slice(j * 512, (j + 1) * 512)
                        ps_v = ps_pool[0].tile([P, 512], F32, tag="pj", bufs=2, name="ps_v")
                        for kd in range(KD):
                            nc.tensor.matmul(
                                ps_v,
                                lhsT=vTt[:, kd, osl],
                                rhs=wvT[:, kd, jsl],
                                start=(kd == 0),
                                stop=(kd == KD - 1),
                            )
                        nc.vector.tensor_tensor(
                            out=vp[:, i, jsl],
                            in0=ps_v,
                            in1=bv_bc[:, jsl],
                            op=mybir.AluOpType.add,
                        )

                    def softmax_tail(h, t, ps):
                        tsl = slice(t * P, (t + 1) * P)
                        p_sb = p_pool.tile([P, SK], BF, name="p_sb")
                        den = den_pool.tile([P, 1], F32, name="den")
                        nc.scalar.activation(
                            out=p_sb, in_=ps, func=AF.Exp, scale=0.125, accum_out=den
                        )
                        rec = den_pool.tile([P, 1], F32, name="rec")
                        nc.vector.reciprocal(out=rec, in_=den)
                        att_sb = att_pool.tile([P, SK], BF, name="att_sb")
                        nc.vector.tensor_scalar_mul(out=att_sb, in0=p_sb, scalar1=rec)
                        wr = nc.gpsimd.dma_start(out=att_d[h, tsl, :], in_=att_sb)
                        att_writes.setdefault(h // HG, []).append(wr.ins)

                    def emit_sp_tile(hp, t):
                        # score+softmax for heads 2hp/2hp+1, query tile t
                        tsl = slice(t * P, (t + 1) * P)
                        ps_a = ps_pool[0].tile([P, SK], F32, tag="s", bufs=3, name="ps_a")
                        ps_b = ps_pool[0].tile([P, SK], F32, tag="s", bufs=3, name="ps_b")
                        for j in range(2):
                            jsl = slice(j * 512, (j + 1) * 512)
                            nc.tensor.matmul(
                                ps_a[:, jsl],
                                lhsT=qpT[0:DK, hp, tsl],
                                rhs=kpT[0:DK, hp, jsl],
                                start=True,
                                stop=True,
                            )
                            nc.tensor.matmul(
                                ps_b[:, jsl],
                                lhsT=qpT[DK : 2 * DK, hp, tsl],
                                rhs=kpT[DK : 2 * DK, hp, jsl],
                                start=True,
                                stop=True,
                            )
                        softmax_tail(2 * hp, t, ps_a)
                        softmax_tail(2 * hp + 1, t, ps_b)

                    attT_tiles = {}
                    att_writes = {}
                    attT_trs = {}
                    attT_order = []
                    attT_consumers = {}

                    def emit_readback(g, engs):
                        for r in range(KT):
                            attT = attT_pool.tile([P, HG * SQ], BF, name="attT")
                            rsl = slice(r * P, (r + 1) * P)
                            src = att_d[HG * g : HG * (g + 1), :, rsl].rearrange(
                                "h q k -> (h q) k"
                            )
                            tr = engs[r].dma_start_transpose(out=attT, in_=src)
                            for w in att_writes[g]:
                                add_dep_helper(tr.ins, w, reason="att DRAM RAW")
                            # WAR: slot re-users wait for the consumers of
                            # the tile that previously occupied this slot
                            idx = len(attT_order)
                            if idx >= 9:
                                for c in attT_consumers.get(attT_order[idx - 9], []):
                                    add_dep_helper(tr.ins, c, reason="attT WAR")
                            attT_order.append((g, r))
                            attT_trs[(g, r)] = tr.ins
                            attT_tiles[(g, r)] = attT

                    def emit_av_group(g):
                        # one PSUM bank per head: concurrent accumulation
                        # chains must not share a bank (start-flag clear
                        # races with the other chain's drain)
                        ps_os = [
                            ps_pool[0].tile(
                                [DK, SQ], F32, tag="o", bufs=HG, name=f"ps_o{u}"
                            )
                            for u in range(HG)
                        ]
                        for r in range(KT):
                            attT = attT_tiles.pop((g, r))
                            for u in range(HG):
                                h = HG * g + u
                                mm = nc.tensor.matmul(
                                    ps_os[u],
                                    lhsT=vp[:, r, h * DK : (h + 1) * DK],
                                    rhs=attT[:, u * SQ : (u + 1) * SQ],
                                    start=(r == 0),
                                    stop=(r == KT - 1),
                                )
                                add_dep_helper(
                                    mm.ins, attT_trs[(g, r)], reason="attT RAW"
                                )
                                attT_consumers.setdefault((g, r), []).append(mm.ins)
                        for w in range(HG // 2):
                            oTb = oTb_pool.tile([P, SQ], BF, name="oTb")
                            nc.vector.tensor_copy(
                                out=oTb[0:DK, :], in_=ps_os[2 * w]
                            )
                            nc.vector.tensor_copy(
                                out=oTb[DK:P, :], in_=ps_os[2 * w + 1]
                            )
                            col0 = (HG * g + 2 * w) * DK
                            for t in range(QT):
                                ptr = ps_pool[0].tile(
                                    [P, P], BF, tag="tr", bufs=2, name="ptr"
                                )
                                nc.tensor.transpose(
                                    ptr, oTb[:, t * P : (t + 1) * P], ident_bf
                                )
                                nc.vector.tensor_copy(
                                    out=out_nat[:, t, col0 : col0 + P], in_=ptr
                                )

                    # ---------- fused main loop ----------
                    # vp is front-loaded (iters 0-3) so the att.V groups can
                    # run inline in the second half; score/softmax tiles are
                    # interleaved between projection units to keep both PE
                    # and ScalarE streams dense.
                    for i in range(OT):
                        if i == 3:
                            emit_readback(0, [nc.sync] * KT)
                        if i == 5:
                            emit_readback(1, [nc.sync] * KT)
                        units = [
                            lambda i=i: emit_qp(i),
                            lambda i=i: emit_kp(i, 0),
                            lambda i=i: emit_kp(i, 1),
                        ]
                        if i < 4:
                            for j in (2 * i, 2 * i + 1):
                                units.append(lambda j=j: emit_vp(j, 0))
                                units.append(lambda j=j: emit_vp(j, 1))
                        if i == 0:
                            for u in units:
                                u()
                            continue
                        ui = 0
                        take = 2 if i < 4 else 1
                        for t in range(QT):
                            emit_sp_tile(i - 1, t)
                            for _ in range(take):
                                if ui < len(units):
                                    units[ui]()
                                    ui += 1
                        while ui < len(units):
                            units[ui]()
                            ui += 1
                    for t in range(QT):
                        emit_sp_tile(OT - 1, t)
                    emit_readback(2, [nc.sync] * KT)
                    emit_readback(3, [nc.sync] * KT)
                    ps_pool[0].release()
                    ps_pool[0] = tc.alloc_tile_pool(name="psB", bufs=2, space="PSUM")
                    for g in range(H // HG):
                        emit_av_group(g)

                    # ---------------- residual + LN ------
                    for t in range(QT):
                        tsl = slice(t * P, (t + 1) * P)
                        x = ln_pool.tile([P, D], F32, name="x", bufs=2)
                        nc.sync.dma_start(out=x, in_=q_f32[tsl, :])
                        nc.vector.tensor_add(out=x, in0=x, in1=out_nat[:, t, :])
                        stats = ln_pool.tile([P, 2, 6], F32, name="stats", bufs=2)
                        for g in range(2):
                            nc.vector.bn_stats(
                                out=stats[:, g, :], in_=x[:, g * 512 : (g + 1) * 512]
                            )
                        mv = ln_pool.tile([P, 2], F32, name="mv", bufs=2)
                        nc.vector.bn_aggr(out=mv, in_=stats)
                        std = ln_pool.tile([P, 1], F32, name="std", bufs=2)
                        nc.scalar.activation(
                            out=std, in_=mv[:, 1:2], func=AF.Sqrt, bias=eps_t
                        )
                        rstd = ln_pool.tile([P, 1], F32, name="rstd", bufs=2)
                        nc.vector.reciprocal(out=rstd, in_=std)
                        nc.vector.tensor_scalar(
                            out=x,
                            in0=x,
                            scalar1=mv[:, 0:1],
                            scalar2=rstd,
                            op0=mybir.AluOpType.subtract,
                            op1=mybir.AluOpType.mult,
                        )
                        nc.vector.tensor_mul(out=x, in0=x, in1=gamma_b)
                        nc.vector.tensor_add(out=x, in0=x, in1=beta_b)
                        nc.gpsimd.dma_start(
                            out=normed_d[t * P : (t + 1) * P, :], in_=x
                        )
                    ps_pool[0].release()
    return nc


_NC_CACHE = None


def _get_nc():
    global _NC_CACHE
    if _NC_CACHE is None:
        nc = bacc.Bacc("TRN2", target_bir_lowering=False, debug=False)
        _emit(nc)
        nc.compile()
        _NC_CACHE = nc
    return _NC_CACHE


def _shard_inputs(q, k, v, Wq, bq, Wk, bk, Wv, bv, gamma, beta):
    bfT = lambda a: np.ascontiguousarray(
        np.asarray(a, dtype=np.float32).T.astype(BF_NP)
    )
    f32 = lambda a: np.ascontiguousarray(np.asarray(a, dtype=np.float32))
    wqT, wkT, wvT = bfT(Wq), bfT(Wk), bfT(Wv)
    bq_f, bk_f, bv_f = (
        f32(bq).reshape(1, D),
        f32(bk).reshape(1, D),
        f32(bv).reshape(1, D),
    )
    gamma_f = f32(gamma).reshape(1, D)
    beta_f = f32(beta).reshape(1, D)
    kT = [bfT(k[b]) for b in range(BS)]
    vT = [bfT(v[b]) for b in range(BS)]
    in_maps = []
    for c in range(N_CORES):
        b = c // 2
        rows = slice((c % 2) * SQ, (c % 2) * SQ + SQ)
        in_maps.append(
            {
                "qT": bfT(q[b, rows, :]),
                "kT": kT[b],
                "vT": vT[b],
                "wqT": wqT,
                "wkT": wkT,
                "wvT": wvT,
                "bq": bq_f,
                "bk": bk_f,
                "bv": bv_f,
                "q_f32": f32(q[b, rows, :]),
                "gamma": gamma_f,
                "beta": beta_f,
            }
        )
    return in_maps


def run_sharded(inputs, trace=False, tmpdir=None):
    """Run the SPMD kernel; returns (normed, att_score, BassKernelResults)."""
    assert int(inputs["head"]) == H
    nc = _get_nc()
    in_maps = _shard_inputs(
        inputs["q"], inputs["k"], inputs["v"],
        inputs["Wq"], inputs["bq"], inputs["Wk"], inputs["bk"],
        inputs["Wv"], inputs["bv"], inputs["gamma"], inputs["beta"],
    )
    res = run_bass_kernel_spmd(
        nc, in_maps, core_ids=list(range(N_CORES)), trace=trace, tmpdir=tmpdir
    )
    normed = np.empty((BS, SEQ, D), np.float32)
    att = np.empty((BS, H, SEQ, SK), np.float32)
    for c in range(N_CORES):
        b = c // 2
        rows = slice((c % 2) * SQ, (c % 2) * SQ + SQ)
        out_c = res.results[c]
        normed[b, rows, :] = out_c["normed"]
        att[b, :, rows, :] = np.asarray(out_c["att"]).astype(np.float32)
    return normed, att, res


def kernel(**inputs):
    normed, att, _ = run_sharded(inputs, trace=False)
    return normed, att


# revision 36
# speedup vs baseline: 1.2034x; 1.2034x over previous
"""Multi-head attention + residual + LayerNorm on 8 TRN2 NeuronCores.

Problem shapes (hardcoded): bs=4, seq=1024, d_model=1024, head=16, dk=64.

Sharding (data parallel over (batch, query-token-half)):
  core c -> batch b = c // 2, query rows [512*(c%2), 512*(c%2)+512).
  Each core computes K/V projections for its batch's full 1024 tokens,
  Q projection for its 512 rows, attention for all 16 heads over its
  512 query rows, and residual+LayerNorm for those rows. No collectives.

Device kernel structure (single fused pipeline):
  - Host feeds q/k/v/W pre-transposed bf16 (contraction dim on
    partitions); all input DMAs are plain contiguous loads, ordered so
    the q/k projections can start as early as possible.
  - Main loop interleaves projection o-tiles with score+softmax work for
    the previous head pair, so ScalarE's exp stream hides under the
    PE's projection matmuls and the PE never idles long enough to lose
    the HAM clock boost.
  - Biases are fused into the PSUM->SBUF copies on the DVE (per-
    partition tensor_scalar for qp^T/kp^T, broadcast tensor_tensor for
    vp) - no PE bias matmuls.
  - scores S = qh^T.T @ kh^T in natural [qt, kt] layout, head pairs
    row-packed on the PE (partitions 0-63 / 64-127); exp on ScalarE
    (scale=1/8 folded, denominator via accum_out); normalize on DVE;
    att written to DRAM bf16 via SWDGE (gpsimd).
  - att read back TRANSPOSED via DMA-transpose in 4-head groups
    ([2048 x 128] -> [128 x 2048], 32 ops, sync ring only), issued as
    soon as each group's att is written so the readbacks overlap the
    main loop; the att.V matmuls run as one dense batch at the end,
    PE-transposed back to natural, then residual + LayerNorm (fp32).
"""

import numpy as np
import ml_dtypes

import concourse.bacc as bacc
import concourse.bass as bass
import concourse.mybir as mybir
import concourse.tile as tile
from concourse.masks import make_identity
from concourse.tile import add_dep_helper
from concourse.bass_utils import run_bass_kernel_spmd

P = 128
BS = 4
SEQ = 1024
D = 1024
H = 16
DK = 64
SQ = 512          # query rows per core
SK = SEQ          # kv rows per core
KD = D // P       # 8 contraction tiles
QT = SQ // P      # 4 query-row tiles
KT = SK // P      # 8 kv-row tiles
OT = D // P       # 8 output-feature tiles
HG = 4            # heads per readback group
EPS = 1e-5

BF = mybir.dt.bfloat16
F32 = mybir.dt.float32
AF = mybir.ActivationFunctionType

N_CORES = 8
BF_NP = ml_dtypes.bfloat16


def _emit(nc):
    """Emit the per-core Tile program."""
    qT_d = nc.dram_tensor("qT", (D, SQ), BF, kind="ExternalInput").ap()
    kT_d = nc.dram_tensor("kT", (D, SK), BF, kind="ExternalInput").ap()
    vT_d = nc.dram_tensor("vT", (D, SK), BF, kind="ExternalInput").ap()
    wqT_d = nc.dram_tensor("wqT", (D, D), BF, kind="ExternalInput").ap()
    wkT_d = nc.dram_tensor("wkT", (D, D), BF, kind="ExternalInput").ap()
    wvT_d = nc.dram_tensor("wvT", (D, D), BF, kind="ExternalInput").ap()
    bq = nc.dram_tensor("bq", (1, D), F32, kind="ExternalInput").ap()
    bk = nc.dram_tensor("bk", (1, D), F32, kind="ExternalInput").ap()
    bv = nc.dram_tensor("bv", (1, D), F32, kind="ExternalInput").ap()
    q_f32 = nc.dram_tensor("q_f32", (SQ, D), F32, kind="ExternalInput").ap()
    gamma = nc.dram_tensor("gamma", (1, D), F32, kind="ExternalInput").ap()
    beta = nc.dram_tensor("beta", (1, D), F32, kind="ExternalInput").ap()

    att_d = nc.dram_tensor("att", (H, SQ, SK), BF, kind="ExternalOutput").ap()
    normed_d = nc.dram_tensor("normed", (SQ, D), F32, kind="ExternalOutput").ap()

    def part3(ap):
        # (KD*P, F) dram view -> [p, kd, F] AP for a single big DMA
        return ap.rearrange("(kd p) f -> p kd f", p=P)

    with tile.TileContext(nc) as tc:
        with (
            tc.tile_pool(name="consts", bufs=1) as consts,
            tc.tile_pool(name="proj", bufs=1) as proj,
        ):
            ps_pool = [tc.alloc_tile_pool(name="psA", bufs=2, space="PSUM")]
            # ---------------- constants (gpsimd queue, tiny) ----------------
            ident_bf = consts.tile([P, P], BF)
            make_identity(nc, ident_bf)
            gamma_b = consts.tile([P, D], F32)
            nc.gpsimd.dma_start(out=gamma_b, in_=gamma.to_broadcast((P, D)))
            beta_b = consts.tile([P, D], F32)
            nc.gpsimd.dma_start(out=beta_b, in_=beta.to_broadcast((P, D)))
            bv_bc = consts.tile([P, D], BF)
            nc.gpsimd.dma_start(out=bv_bc, in_=bv.to_broadcast((P, D)))
            eps_t = consts.tile([P, 1], F32)
            nc.vector.memset(eps_t, EPS)
            # per-partition bias columns: bq_col[p, i] = bq[i*128 + p]
            bq_col = consts.tile([P, KD], F32)
            nc.gpsimd.dma_start(out=bq_col, in_=bq.rearrange("a (i p) -> p (a i)", p=P))
            bk_col = consts.tile([P, KD], F32)
            nc.gpsimd.dma_start(out=bk_col, in_=bk.rearrange("a (i p) -> p (a i)", p=P))

            # ---------------- input loads (pre-transposed on host) ----------
            # ordered so qp/kp (and the first score matmuls) start earliest
            with tc.tile_pool(name="inputs", bufs=1) as inp:
                wqT = inp.tile([P, KD, D], BF)
                wkT = inp.tile([P, KD, D], BF)
                wvT = inp.tile([P, KD, D], BF)
                kTt = inp.tile([P, KD, SK], BF)
                vTt = inp.tile([P, KD, SK], BF)
                qTt = inp.tile([P, KD, SQ], BF)
                nc.sync.dma_start(out=qTt, in_=part3(qT_d))
                nc.scalar.dma_start(out=wqT[:, :, :512], in_=part3(wqT_d)[:, :, :512])
                nc.sync.dma_start(out=kTt, in_=part3(kT_d))
                nc.scalar.dma_start(out=wkT[:, :, :512], in_=part3(wkT_d)[:, :, :512])
                nc.scalar.dma_start(out=wvT[:, :, :512], in_=part3(wvT_d)[:, :, :512])
                nc.sync.dma_start(out=vTt, in_=part3(vT_d))
                nc.scalar.dma_start(out=wqT[:, :, 512:], in_=part3(wqT_d)[:, :, 512:])
                nc.scalar.dma_start(out=wkT[:, :, 512:], in_=part3(wkT_d)[:, :, 512:])
                nc.scalar.dma_start(out=wvT[:, :, 512:], in_=part3(wvT_d)[:, :, 512:])

                qpT = proj.tile([P, OT, SQ], BF)  # qp^T[o, qt]
                kpT = proj.tile([P, OT, SK], BF)  # kp^T[o, kt]
                vp = proj.tile([P, KT, D], BF)    # vp[kt, o]

                with (
                    tc.tile_pool(name="p_sb", bufs=4) as p_pool,
                    tc.tile_pool(name="att_sb", bufs=3) as att_pool,
                    tc.tile_pool(name="attT", bufs=8) as attT_pool,
                    tc.tile_pool(name="den", bufs=12) as den_pool,
                    tc.tile_pool(name="oTb", bufs=2) as oTb_pool,
                    tc.tile_pool(name="ln", bufs=1) as ln_pool,
                ):
                    out_nat = proj.tile([P, QT, D], BF)

                    def emit_qp(i):
                        osl = slice(i * P, (i + 1) * P)
                        ps_q = ps_pool[0].tile([P, 512], F32, tag="pj", bufs=2, name="ps_q")
                        for kd in range(KD):
                            nc.tensor.matmul(
                                ps_q,
                                lhsT=wqT[:, kd, osl],
                                rhs=qTt[:, kd, :],
                                start=(kd == 0),
                                stop=(kd == KD - 1),
                            )
                        nc.vector.tensor_scalar_add(
                            out=qpT[:, i, :],
                            in0=ps_q,
                            scalar1=bq_col[:, i : i + 1],
                        )

                    def emit_kp(i, j):
                        osl = slice(i * P, (i + 1) * P)
                        jsl = slice(j * 512, (j + 1) * 512)
                        ps_k = ps_pool[0].tile([P, 512], F32, tag="pj", bufs=2, name="ps_k")
                        for kd in range(KD):
                            nc.tensor.matmul(
                                ps_k,
                                lhsT=wkT[:, kd, osl],
                                rhs=kTt[:, kd, jsl],
                                start=(kd == 0),
                                stop=(kd == KD - 1),
                            )
                        nc.vector.tensor_scalar_add(
                            out=kpT[:, i, jsl],
                            in0=ps_k,
                            scalar1=bk_col[:, i : i + 1],
                        )

                    def emit_vp(i, j):
                        osl = slice(i * P, (i + 1) * P)
                        jsl = slice(j * 512, (j + 1) * 512)
                        ps_v = ps_pool[0].tile([P, 512], F32, tag="pj", bufs=2, name="ps_v")
                        for kd in range(KD):
                            nc.tensor.matmul(
                                ps_v,
                                lhsT=vTt[:, kd, osl],
                                rhs=wvT[:, kd, jsl],
                                start=(kd == 0),
                                stop=(kd == KD - 1),
                            )
                        nc.vector.tensor_tensor(
                            out=vp[:, i, jsl],
                            in0=ps_v,
                            in1=bv_bc[:, jsl],
                            op=mybir.AluOpType.add,
                        )

                    def softmax_tail(h, t, ps):
                        tsl = slice(t * P, (t + 1) * P)
                        p_sb = p_pool.tile([P, SK], BF, name="p_sb")
                        den = den_pool.tile([P, 1], F32, name="den")
                        nc.scalar.activation(
                            out=p_sb, in_=ps, func=AF.Exp, scale=0.125, accum_out=den
                        )
                        rec = den_pool.tile([P, 1], F32, name="rec")
                        nc.vector.reciprocal(out=rec, in_=den)
                        att_sb = att_pool.tile([P, SK], BF, name="att_sb")
                        nc.vector.tensor_scalar_mul(out=att_sb, in0=p_sb, scalar1=rec)
                        wr = nc.sync.dma_start(out=att_d[h, tsl, :], in_=att_sb)
                        att_writes.setdefault(h // HG, []).append(wr.ins)

                    def emit_sp_tile(hp, t):
                        # score+softmax for heads 2hp/2hp+1, query tile t
                        tsl = slice(t * P, (t + 1) * P)
                        ps_a = ps_pool[0].tile([P, SK], F32, tag="s", bufs=3, name="ps_a")
                        ps_b = ps_pool[0].tile([P, SK], F32, tag="s", bufs=3, name="ps_b")
                        for j in range(2):
                            jsl = slice(j * 512, (j + 1) * 512)
                            nc.tensor.matmul(
                                ps_a[:, jsl],
                                lhsT=qpT[0:DK, hp, tsl],
                                rhs=kpT[0:DK, hp, jsl],
                                start=True,
                                stop=True,
                            )
                            nc.tensor.matmul(
                                ps_b[:, jsl],
                                lhsT=qpT[DK : 2 * DK, hp, tsl],
                                rhs=kpT[DK : 2 * DK, hp, jsl],
                                start=True,
                                stop=True,
                            )
                        softmax_tail(2 * hp, t, ps_a)
                        softmax_tail(2 * hp + 1, t, ps_b)

                    attT_tiles = {}
                    att_writes = {}
                    attT_trs = {}
                    attT_order = []
                    attT_consumers = {}

                    def emit_readback(g, engs):
                        for r in range(KT):
                            attT = attT_pool.tile([P, HG * SQ], BF, name="attT")
                            rsl = slice(r * P, (r + 1) * P)
                            src = att_d[HG * g : HG * (g + 1), :, rsl].rearrange(
                                "h q k -> (h q) k"
                            )
                            tr = engs[r].dma_start_transpose(out=attT, in_=src)
                            for w in att_writes[g]:
                                add_dep_helper(tr.ins, w, reason="att DRAM RAW")
                            # WAR: slot re-users wait for the consumers of
                            # the tile that previously occupied this slot
                            idx = len(attT_order)
                            if idx >= 8:
                                for c in attT_consumers.get(attT_order[idx - 8], []):
                                    add_dep_helper(tr.ins, c, reason="attT WAR")
                            attT_order.append((g, r))
                            attT_trs[(g, r)] = tr.ins
                            attT_tiles[(g, r)] = attT

                    def emit_av_group(g):
                        # one PSUM bank per head: concurrent accumulation
                        # chains must not share a bank (start-flag clear
                        # races with the other chain's drain)
                        ps_os = [
                            ps_pool[0].tile(
                                [DK, SQ], F32, tag="o", bufs=HG, name=f"ps_o{u}"
                            )
                            for u in range(HG)
                        ]
                        for r in range(KT):
                            attT = attT_tiles.pop((g, r))
                            for u in range(HG):
                                h = HG * g + u
                                mm = nc.tensor.matmul(
                                    ps_os[u],
                                    lhsT=vp[:, r, h * DK : (h + 1) * DK],
                                    rhs=attT[:, u * SQ : (u + 1) * SQ],
                                    start=(r == 0),
                                    stop=(r == KT - 1),
                                )
                                add_dep_helper(
                                    mm.ins, attT_trs[(g, r)], reason="attT RAW"
                                )
                                attT_consumers.setdefault((g, r), []).append(mm.ins)
                        for w in range(HG // 2):
                            oTb = oTb_pool.tile([P, SQ], BF, name="oTb")
                            nc.vector.tensor_copy(
                                out=oTb[0:DK, :], in_=ps_os[2 * w]
                            )
                            nc.vector.tensor_copy(
                                out=oTb[DK:P, :], in_=ps_os[2 * w + 1]
                            )
                            col0 = (HG * g + 2 * w) * DK
                            for t in range(QT):
                                ptr = ps_pool[0].tile(
                                    [P, P], BF, tag="tr", bufs=2, name="ptr"
                                )
                                nc.tensor.transpose(
                                    ptr, oTb[:, t * P : (t + 1) * P], ident_bf
                                )
                                nc.vector.tensor_copy(
                                    out=out_nat[:, t, col0 : col0 + P], in_=ptr
                                )

                    # ---------- fused main loop ----------
                    # vp is front-loaded (iters 0-3) so the att.V groups can
                    # run inline in the second half; score/softmax tiles are
                    # interleaved between projection units to keep both PE
                    # and ScalarE streams dense.
                    for i in range(OT):
                        if i == 3:
                            emit_readback(0, [nc.sync] * KT)
                        if i == 5:
                            emit_readback(1, [nc.sync] * KT)
                        units = [
                            lambda i=i: emit_qp(i),
                            lambda i=i: emit_kp(i, 0),
                            lambda i=i: emit_kp(i, 1),
                        ]
                        if i < 4:
                            for j in (2 * i, 2 * i + 1):
                                units.append(lambda j=j: emit_vp(j, 0))
                                units.append(lambda j=j: emit_vp(j, 1))
                        if i == 0:
                            for u in units:
                                u()
                            continue
                        ui = 0
                        take = 2 if i < 4 else 1
                        for t in range(QT):
                            emit_sp_tile(i - 1, t)
                            for _ in range(take):
                                if ui < len(units):
                                    units[ui]()
                                    ui += 1
                        while ui < len(units):
                            units[ui]()
                            ui += 1
                    for t in range(QT):
                        emit_sp_tile(OT - 1, t)
                    emit_readback(2, [nc.sync] * KT)
                    emit_readback(3, [nc.sync] * KT)
                    ps_pool[0].release()
                    ps_pool[0] = tc.alloc_tile_pool(name="psB", bufs=2, space="PSUM")
                    for g in range(H // HG):
                        emit_av_group(g)

                    # ---------------- residual + LN ------
                    for t in range(QT):
                        tsl = slice(t * P, (t + 1) * P)
                        x = ln_pool.tile([P, D], F32, name="x", bufs=2)
                        nc.sync.dma_start(out=x, in_=q_f32[tsl, :])
                        nc.vector.tensor_add(out=x, in0=x, in1=out_nat[:, t, :])
                        stats = ln_pool.tile([P, 2, 6], F32, name="stats", bufs=2)
                        for g in range(2):
                            nc.vector.bn_stats(
                                out=stats[:, g, :], in_=x[:, g * 512 : (g + 1) * 512]
                            )
                        mv = ln_pool.tile([P, 2], F32, name="mv", bufs=2)
                        nc.vector.bn_aggr(out=mv, in_=stats)
                        std = ln_pool.tile([P, 1], F32, name="std", bufs=2)
                        nc.scalar.activation(
                            out=std, in_=mv[:, 1:2], func=AF.Sqrt, bias=eps_t
                        )
                        rstd = ln_pool.tile([P, 1], F32, name="rstd", bufs=2)
                        nc.vector.reciprocal(out=rstd, in_=std)
                        nc.vector.tensor_scalar(
                            out=x,
                            in0=x,
                            scalar1=mv[:, 0:1],
                            scalar2=rstd,
                            op0=mybir.AluOpType.subtract,
                            op1=mybir.AluOpType.mult,
                        )
                        nc.vector.tensor_mul(out=x, in0=x, in1=gamma_b)
                        nc.vector.tensor_add(out=x, in0=x, in1=beta_b)
                        nc.gpsimd.dma_start(
                            out=normed_d[t * P : (t + 1) * P, :], in_=x
                        )
                    ps_pool[0].release()
    return nc


_NC_CACHE = None


def _get_nc():
    global _NC_CACHE
    if _NC_CACHE is None:
        nc = bacc.Bacc("TRN2", target_bir_lowering=False, debug=False)
        _emit(nc)
        nc.compile()
        _NC_CACHE = nc
    return _NC_CACHE


def _shard_inputs(q, k, v, Wq, bq, Wk, bk, Wv, bv, gamma, beta):
    bfT = lambda a: np.ascontiguousarray(
        np.asarray(a, dtype=np.float32).T.astype(BF_NP)
    )
    f32 = lambda a: np.ascontiguousarray(np.asarray(a, dtype=np.float32))
    wqT, wkT, wvT = bfT(Wq), bfT(Wk), bfT(Wv)
    bq_f, bk_f, bv_f = (
        f32(bq).reshape(1, D),
        f32(bk).reshape(1, D),
        f32(bv).reshape(1, D),
    )
    gamma_f = f32(gamma).reshape(1, D)
    beta_f = f32(beta).reshape(1, D)
    kT = [bfT(k[b]) for b in range(BS)]
    vT = [bfT(v[b]) for b in range(BS)]
    in_maps = []
    for c in range(N_CORES):
        b = c // 2
        rows = slice((c % 2) * SQ, (c % 2) * SQ + SQ)
        in_maps.append(
            {
                "qT": bfT(q[b, rows, :]),
                "kT": kT[b],
                "vT": vT[b],
                "wqT": wqT,
                "wkT": wkT,
                "wvT": wvT,
                "bq": bq_f,
                "bk": bk_f,
                "bv": bv_f,
                "q_f32": f32(q[b, rows, :]),
                "gamma": gamma_f,
                "beta": beta_f,
            }
        )
    return in_maps


def run_sharded(inputs, trace=False, tmpdir=None):
    """Run the SPMD kernel; returns (normed, att_score, BassKernelResults)."""
    assert int(inputs["head"]) == H
    nc = _get_nc()
    in_maps = _shard_inputs(
        inputs["q"], inputs["k"], inputs["v"],
        inputs["Wq"], inputs["bq"], inputs["Wk"], inputs["bk"],
        inputs["Wv"], inputs["bv"], inputs["gamma"], inputs["beta"],
    )
    res = run_bass_kernel_spmd(
        nc, in_maps, core_ids=list(range(N_CORES)), trace=trace, tmpdir=tmpdir
    )
    normed = np.empty((BS, SEQ, D), np.float32)
    att = np.empty((BS, H, SEQ, SK), np.float32)
    for c in range(N_CORES):
        b = c // 2
        rows = slice((c % 2) * SQ, (c % 2) * SQ + SQ)
        out_c = res.results[c]
        normed[b, rows, :] = out_c["normed"]
        att[b, :, rows, :] = np.asarray(out_c["att"]).astype(np.float32)
    return normed, att, res


def kernel(**inputs):
    normed, att, _ = run_sharded(inputs, trace=False)
    return normed, att


# revision 37
# speedup vs baseline: 1.2156x; 1.0101x over previous
"""Multi-head attention + residual + LayerNorm on 8 TRN2 NeuronCores.

Problem shapes (hardcoded): bs=4, seq=1024, d_model=1024, head=16, dk=64.

Sharding (data parallel over (batch, query-token-half)):
  core c -> batch b = c // 2, query rows [512*(c%2), 512*(c%2)+512).
  Each core computes K/V projections for its batch's full 1024 tokens,
  Q projection for its 512 rows, attention for all 16 heads over its
  512 query rows, and residual+LayerNorm for those rows. No collectives.

Device kernel structure (single fused pipeline):
  - Host feeds q/k/v/W pre-transposed bf16 (contraction dim on
    partitions); all input DMAs are plain contiguous loads, ordered so
    the q/k projections can start as early as possible.
  - Main loop interleaves projection o-tiles with score+softmax work for
    the previous head pair, so ScalarE's exp stream hides under the
    PE's projection matmuls and the PE never idles long enough to lose
    the HAM clock boost.
  - Biases are fused into the PSUM->SBUF copies on the DVE (per-
    partition tensor_scalar for qp^T/kp^T, broadcast tensor_tensor for
    vp) - no PE bias matmuls.
  - scores S = qh^T.T @ kh^T in natural [qt, kt] layout, head pairs
    row-packed on the PE (partitions 0-63 / 64-127); exp on ScalarE
    (scale=1/8 folded, denominator via accum_out); normalize on DVE;
    att written to DRAM bf16 via SWDGE (gpsimd).
  - att read back TRANSPOSED via DMA-transpose in 4-head groups
    ([2048 x 128] -> [128 x 2048], 32 ops, sync ring only), issued as
    soon as each group's att is written so the readbacks overlap the
    main loop; the att.V matmuls run as one dense batch at the end,
    PE-transposed back to natural, then residual + LayerNorm (fp32).
"""

import numpy as np
import ml_dtypes

import concourse.bacc as bacc
import concourse.bass as bass
import concourse.mybir as mybir
import concourse.tile as tile
from concourse.masks import make_identity
from concourse.tile import add_dep_helper
from concourse.bass_utils import run_bass_kernel_spmd

P = 128
BS = 4
SEQ = 1024
D = 1024
H = 16
DK = 64
SQ = 512          # query rows per core
SK = SEQ          # kv rows per core
KD = D // P       # 8 contraction tiles
QT = SQ // P      # 4 query-row tiles
KT = SK // P      # 8 kv-row tiles
OT = D // P       # 8 output-feature tiles
HG = 4            # heads per readback group
EPS = 1e-5

BF = mybir.dt.bfloat16
F32 = mybir.dt.float32
AF = mybir.ActivationFunctionType

N_CORES = 8
BF_NP = ml_dtypes.bfloat16


def _emit(nc):
    """Emit the per-core Tile program."""
    qT_d = nc.dram_tensor("qT", (D, SQ), BF, kind="ExternalInput").ap()
    kT_d = nc.dram_tensor("kT", (D, SK), BF, kind="ExternalInput").ap()
    vT_d = nc.dram_tensor("vT", (D, SK), BF, kind="ExternalInput").ap()
    wqT_d = nc.dram_tensor("wqT", (D, D), BF, kind="ExternalInput").ap()
    wkT_d = nc.dram_tensor("wkT", (D, D), BF, kind="ExternalInput").ap()
    wvT_d = nc.dram_tensor("wvT", (D, D), BF, kind="ExternalInput").ap()
    bq = nc.dram_tensor("bq", (1, D), F32, kind="ExternalInput").ap()
    bk = nc.dram_tensor("bk", (1, D), F32, kind="ExternalInput").ap()
    bv = nc.dram_tensor("bv", (1, D), F32, kind="ExternalInput").ap()
    q_f32 = nc.dram_tensor("q_f32", (SQ, D), F32, kind="ExternalInput").ap()
    gamma = nc.dram_tensor("gamma", (1, D), F32, kind="ExternalInput").ap()
    beta = nc.dram_tensor("beta", (1, D), F32, kind="ExternalInput").ap()

    att_d = nc.dram_tensor("att", (H, SQ, SK), BF, kind="ExternalOutput").ap()
    normed_d = nc.dram_tensor("normed", (SQ, D), F32, kind="ExternalOutput").ap()

    def part3(ap):
        # (KD*P, F) dram view -> [p, kd, F] AP for a single big DMA
        return ap.rearrange("(kd p) f -> p kd f", p=P)

    with tile.TileContext(nc) as tc:
        with (
            tc.tile_pool(name="consts", bufs=1) as consts,
            tc.tile_pool(name="proj", bufs=1) as proj,
        ):
            ps_pool = [tc.alloc_tile_pool(name="psA", bufs=2, space="PSUM")]
            # ---------------- constants (gpsimd queue, tiny) ----------------
            ident_bf = consts.tile([P, P], BF)
            make_identity(nc, ident_bf)
            gamma_b = consts.tile([P, D], F32)
            nc.gpsimd.dma_start(out=gamma_b, in_=gamma.to_broadcast((P, D)))
            beta_b = consts.tile([P, D], F32)
            nc.gpsimd.dma_start(out=beta_b, in_=beta.to_broadcast((P, D)))
            bv_bc = consts.tile([P, D], BF)
            nc.gpsimd.dma_start(out=bv_bc, in_=bv.to_broadcast((P, D)))
            eps_t = consts.tile([P, 1], F32)
            nc.vector.memset(eps_t, EPS)
            # per-partition bias columns: bq_col[p, i] = bq[i*128 + p]
            bq_col = consts.tile([P, KD], F32)
            nc.gpsimd.dma_start(out=bq_col, in_=bq.rearrange("a (i p) -> p (a i)", p=P))
            bk_col = consts.tile([P, KD], F32)
            nc.gpsimd.dma_start(out=bk_col, in_=bk.rearrange("a (i p) -> p (a i)", p=P))

            # ---------------- input loads (pre-transposed on host) ----------
            # ordered so qp/kp (and the first score matmuls) start earliest
            with tc.tile_pool(name="inputs", bufs=1) as inp:
                wqT = inp.tile([P, KD, D], BF)
                wkT = inp.tile([P, KD, D], BF)
                wvT = inp.tile([P, KD, D], BF)
                kTt = inp.tile([P, KD, SK], BF)
                vTt = inp.tile([P, KD, SK], BF)
                qTt = inp.tile([P, KD, SQ], BF)
                nc.sync.dma_start(out=qTt, in_=part3(qT_d))
                nc.scalar.dma_start(out=wqT[:, :, :512], in_=part3(wqT_d)[:, :, :512])
                nc.sync.dma_start(out=kTt, in_=part3(kT_d))
                nc.scalar.dma_start(out=wkT[:, :, :512], in_=part3(wkT_d)[:, :, :512])
                nc.scalar.dma_start(out=wvT[:, :, :512], in_=part3(wvT_d)[:, :, :512])
                nc.sync.dma_start(out=vTt, in_=part3(vT_d))
                nc.scalar.dma_start(out=wqT[:, :, 512:], in_=part3(wqT_d)[:, :, 512:])
                nc.scalar.dma_start(out=wkT[:, :, 512:], in_=part3(wkT_d)[:, :, 512:])
                nc.scalar.dma_start(out=wvT[:, :, 512:], in_=part3(wvT_d)[:, :, 512:])

                qpT = proj.tile([P, OT, SQ], BF)  # qp^T[o, qt]
                kpT = proj.tile([P, OT, SK], BF)  # kp^T[o, kt]
                vp = proj.tile([P, KT, D], BF)    # vp[kt, o]

                with (
                    tc.tile_pool(name="p_sb", bufs=4) as p_pool,
                    tc.tile_pool(name="att_sb", bufs=3) as att_pool,
                    tc.tile_pool(name="attT", bufs=8) as attT_pool,
                    tc.tile_pool(name="den", bufs=12) as den_pool,
                    tc.tile_pool(name="oTb", bufs=2) as oTb_pool,
                    tc.tile_pool(name="ln", bufs=1) as ln_pool,
                ):
                    out_nat = proj.tile([P, QT, D], BF)

                    def emit_qp(i):
                        osl = slice(i * P, (i + 1) * P)
                        ps_q = ps_pool[0].tile([P, 512], F32, tag="pj", bufs=2, name="ps_q")
                        for kd in range(KD):
                            nc.tensor.matmul(
                                ps_q,
                                lhsT=wqT[:, kd, osl],
                                rhs=qTt[:, kd, :],
                                start=(kd == 0),
                                stop=(kd == KD - 1),
                            )
                        nc.vector.tensor_scalar_add(
                            out=qpT[:, i, :],
                            in0=ps_q,
                            scalar1=bq_col[:, i : i + 1],
                        )

                    def emit_kp(i, j):
                        osl = slice(i * P, (i + 1) * P)
                        jsl = slice(j * 512, (j + 1) * 512)
                        ps_k = ps_pool[0].tile([P, 512], F32, tag="pj", bufs=2, name="ps_k")
                        for kd in range(KD):
                            nc.tensor.matmul(
                                ps_k,
                                lhsT=wkT[:, kd, osl],
                                rhs=kTt[:, kd, jsl],
                                start=(kd == 0),
                                stop=(kd == KD - 1),
                            )
                        nc.vector.tensor_scalar_add(
                            out=kpT[:, i, jsl],
                            in0=ps_k,
                            scalar1=bk_col[:, i : i + 1],
                        )

                    def emit_vp(i, j):
                        osl = slice(i * P, (i + 1) * P)
                        jsl = slice(j * 512, (j + 1) * 512)
                        ps_v = ps_pool[0].tile([P, 512], F32, tag="pj", bufs=2, name="ps_v")
                        for kd in range(KD):
                            nc.tensor.matmul(
                                ps_v,
                                lhsT=vTt[:, kd, osl],
                                rhs=wvT[:, kd, jsl],
                                start=(kd == 0),
                                stop=(kd == KD - 1),
                            )
                        nc.vector.tensor_tensor(
                            out=vp[:, i, jsl],
                            in0=ps_v,
                            in1=bv_bc[:, jsl],
                            op=mybir.AluOpType.add,
                        )

                    def softmax_tail(h, t, ps):
                        tsl = slice(t * P, (t + 1) * P)
                        p_sb = p_pool.tile([P, SK], BF, name="p_sb")
                        den = den_pool.tile([P, 1], F32, name="den")
                        nc.scalar.activation(
                            out=p_sb, in_=ps, func=AF.Exp, scale=0.125, accum_out=den
                        )
                        rec = den_pool.tile([P, 1], F32, name="rec")
                        nc.vector.reciprocal(out=rec, in_=den)
                        att_sb = att_pool.tile([P, SK], BF, name="att_sb")
                        nc.vector.tensor_scalar_mul(out=att_sb, in0=p_sb, scalar1=rec)
                        wr = nc.sync.dma_start(out=att_d[h, tsl, :], in_=att_sb)
                        att_writes.setdefault(h // HG, []).append(wr.ins)

                    def emit_sp_tile(hp, t):
                        # score+softmax for heads 2hp/2hp+1, query tile t
                        tsl = slice(t * P, (t + 1) * P)
                        ps_a = ps_pool[0].tile([P, SK], F32, tag="s", bufs=3, name="ps_a")
                        ps_b = ps_pool[0].tile([P, SK], F32, tag="s", bufs=3, name="ps_b")
                        for j in range(2):
                            jsl = slice(j * 512, (j + 1) * 512)
                            nc.tensor.matmul(
                                ps_a[:, jsl],
                                lhsT=qpT[0:DK, hp, tsl],
                                rhs=kpT[0:DK, hp, jsl],
                                start=True,
                                stop=True,
                            )
                            nc.tensor.matmul(
                                ps_b[:, jsl],
                                lhsT=qpT[DK : 2 * DK, hp, tsl],
                                rhs=kpT[DK : 2 * DK, hp, jsl],
                                start=True,
                                stop=True,
                            )
                        softmax_tail(2 * hp, t, ps_a)
                        softmax_tail(2 * hp + 1, t, ps_b)

                    attT_tiles = {}
                    att_writes = {}
                    attT_trs = {}
                    attT_order = []
                    attT_consumers = {}

                    def emit_readback(g, engs):
                        for r in range(KT):
                            attT = attT_pool.tile([P, HG * SQ], BF, name="attT")
                            rsl = slice(r * P, (r + 1) * P)
                            src = att_d[HG * g : HG * (g + 1), :, rsl].rearrange(
                                "h q k -> (h q) k"
                            )
                            tr = engs[r].dma_start_transpose(out=attT, in_=src)
                            for w in att_writes[g]:
                                add_dep_helper(tr.ins, w, reason="att DRAM RAW")
                            # WAR: slot re-users wait for the consumers of
                            # the tile that previously occupied this slot
                            idx = len(attT_order)
                            if idx >= 8:
                                for c in attT_consumers.get(attT_order[idx - 8], []):
                                    add_dep_helper(tr.ins, c, reason="attT WAR")
                            attT_order.append((g, r))
                            attT_trs[(g, r)] = tr.ins
                            attT_tiles[(g, r)] = attT

                    def emit_av_group(g):
                        # one PSUM bank per head: concurrent accumulation
                        # chains must not share a bank (start-flag clear
                        # races with the other chain's drain)
                        ps_os = [
                            ps_pool[0].tile(
                                [DK, SQ], F32, tag="o", bufs=HG, name=f"ps_o{u}"
                            )
                            for u in range(HG)
                        ]
                        for r in range(KT):
                            attT = attT_tiles.pop((g, r))
                            for u in range(HG):
                                h = HG * g + u
                                mm = nc.tensor.matmul(
                                    ps_os[u],
                                    lhsT=vp[:, r, h * DK : (h + 1) * DK],
                                    rhs=attT[:, u * SQ : (u + 1) * SQ],
                                    start=(r == 0),
                                    stop=(r == KT - 1),
                                )
                                add_dep_helper(
                                    mm.ins, attT_trs[(g, r)], reason="attT RAW"
                                )
                                attT_consumers.setdefault((g, r), []).append(mm.ins)
                        for w in range(HG // 2):
                            oTb = oTb_pool.tile([P, SQ], BF, name="oTb")
                            nc.vector.tensor_copy(
                                out=oTb[0:DK, :], in_=ps_os[2 * w]
                            )
                            nc.vector.tensor_copy(
                                out=oTb[DK:P, :], in_=ps_os[2 * w + 1]
                            )
                            col0 = (HG * g + 2 * w) * DK
                            for t in range(QT):
                                ptr = ps_pool[0].tile(
                                    [P, P], BF, tag="tr", bufs=2, name="ptr"
                                )
                                nc.tensor.transpose(
                                    ptr, oTb[:, t * P : (t + 1) * P], ident_bf
                                )
                                nc.vector.tensor_copy(
                                    out=out_nat[:, t, col0 : col0 + P], in_=ptr
                                )

                    # ---------- fused main loop ----------
                    # vp is front-loaded (iters 0-3) so the att.V groups can
                    # run inline in the second half; score/softmax tiles are
                    # interleaved between projection units to keep both PE
                    # and ScalarE streams dense.
                    for i in range(OT):
                        if i == 3:
                            emit_readback(0, [nc.sync] * KT)
                        if i == 5:
                            emit_readback(1, [nc.sync] * KT)
                        units = [
                            lambda i=i: emit_qp(i),
                            lambda i=i: emit_kp(i, 0),
                            lambda i=i: emit_kp(i, 1),
                        ]
                        if i < 4:
                            for j in (2 * i, 2 * i + 1):
                                units.append(lambda j=j: emit_vp(j, 0))
                                units.append(lambda j=j: emit_vp(j, 1))
                        if i == 0:
                            for u in units:
                                u()
                            continue
                        ui = 0
                        take = 2 if i < 4 else 1
                        for t in range(QT):
                            emit_sp_tile(i - 1, t)
                            for _ in range(take):
                                if ui < len(units):
                                    units[ui]()
                                    ui += 1
                        while ui < len(units):
                            units[ui]()
                            ui += 1
                    for t in range(QT):
                        emit_sp_tile(OT - 1, t)
                    emit_readback(2, [nc.sync] * KT)
                    emit_readback(3, [nc.sync] * KT)
                    ps_pool[0].release()
                    ps_pool[0] = tc.alloc_tile_pool(name="psB", bufs=2, space="PSUM")
                    for g in range(H // HG):
                        emit_av_group(g)

                    # ---------------- residual + LN ------
                    for t in range(QT):
                        tsl = slice(t * P, (t + 1) * P)
                        x = ln_pool.tile([P, D], F32, name="x", bufs=2)
                        nc.sync.dma_start(out=x, in_=q_f32[tsl, :])
                        nc.vector.tensor_add(out=x, in0=x, in1=out_nat[:, t, :])
                        stats = ln_pool.tile([P, 2, 6], F32, name="stats", bufs=2)
                        for g in range(2):
                            nc.vector.bn_stats(
                                out=stats[:, g, :], in_=x[:, g * 512 : (g + 1) * 512]
                            )
                        mv = ln_pool.tile([P, 2], F32, name="mv", bufs=2)
                        nc.vector.bn_aggr(out=mv, in_=stats)
                        std = ln_pool.tile([P, 1], F32, name="std", bufs=2)
                        nc.scalar.activation(
                            out=std, in_=mv[:, 1:2], func=AF.Sqrt, bias=eps_t
                        )
                        rstd = ln_pool.tile([P, 1], F32, name="rstd", bufs=2)
                        nc.vector.reciprocal(out=rstd, in_=std)
                        nc.vector.tensor_scalar(
                            out=x,
                            in0=x,
                            scalar1=mv[:, 0:1],
                            scalar2=rstd,
                            op0=mybir.AluOpType.subtract,
                            op1=mybir.AluOpType.mult,
                        )
                        nc.vector.tensor_mul(out=x, in0=x, in1=gamma_b)
                        nc.vector.tensor_add(out=x, in0=x, in1=beta_b)
                        nc.sync.dma_start(
                            out=normed_d[t * P : (t + 1) * P, :], in_=x
                        )
                    ps_pool[0].release()
    return nc


_NC_CACHE = None


def _get_nc():
    global _NC_CACHE
    if _NC_CACHE is None:
        nc = bacc.Bacc("TRN2", target_bir_lowering=False, debug=False)
        _emit(nc)
        nc.compile()
        _NC_CACHE = nc
    return _NC_CACHE


def _shard_inputs(q, k, v, Wq, bq, Wk, bk, Wv, bv, gamma, beta):
    bfT = lambda a: np.ascontiguousarray(
        np.asarray(a, dtype=np.float32).T.astype(BF_NP)
    )
    f32 = lambda a: np.ascontiguousarray(np.asarray(a, dtype=np.float32))
    wqT, wkT, wvT = bfT(Wq), bfT(Wk), bfT(Wv)
    bq_f, bk_f, bv_f = (
        f32(bq).reshape(1, D),
        f32(bk).reshape(1, D),
        f32(bv).reshape(1, D),
    )
    gamma_f = f32(gamma).reshape(1, D)
    beta_f = f32(beta).reshape(1, D)
    kT = [bfT(k[b]) for b in range(BS)]
    vT = [bfT(v[b]) for b in range(BS)]
    in_maps = []
    for c in range(N_CORES):
        b = c // 2
        rows = slice((c % 2) * SQ, (c % 2) * SQ + SQ)
        in_maps.append(
            {
                "qT": bfT(q[b, rows, :]),
                "kT": kT[b],
                "vT": vT[b],
                "wqT": wqT,
                "wkT": wkT,
                "wvT": wvT,
                "bq": bq_f,
                "bk": bk_f,
                "bv": bv_f,
                "q_f32": f32(q[b, rows, :]),
                "gamma": gamma_f,
                "beta": beta_f,
            }
        )
    return in_maps


def run_sharded(inputs, trace=False, tmpdir=None):
    """Run the SPMD kernel; returns (normed, att_score, BassKernelResults)."""
    assert int(inputs["head"]) == H
    nc = _get_nc()
    in_maps = _shard_inputs(
        inputs["q"], inputs["k"], inputs["v"],
        inputs["Wq"], inputs["bq"], inputs["Wk"], inputs["bk"],
        inputs["Wv"], inputs["bv"], inputs["gamma"], inputs["beta"],
    )
    res = run_bass_kernel_spmd(
        nc, in_maps, core_ids=list(range(N_CORES)), trace=trace, tmpdir=tmpdir
    )
    normed = np.empty((BS, SEQ, D), np.float32)
    att = np.empty((BS, H, SEQ, SK), np.float32)
    for c in range(N_CORES):
        b = c // 2
        rows = slice((c % 2) * SQ, (c % 2) * SQ + SQ)
        out_c = res.results[c]
        normed[b, rows, :] = out_c["normed"]
        att[b, :, rows, :] = np.asarray(out_c["att"]).astype(np.float32)
    return normed, att, res


def kernel(**inputs):
    normed, att, _ = run_sharded(inputs, trace=False)
    return normed, att
